# revision 4
# baseline (speedup 1.0000x reference)
"""GCN-with-edge-features kernel for 8 Trainium2 cores.

Strategy: the per-edge weight matrices theta = relu(ea@Wa+ba)@Wb+bb
depend ONLY on the 16-bit edge attribute, and E=100k random edges hit
only ~51.3k distinct attribute values. We compute theta once per
UNIQUE attribute (0.51x the dominant GEMM FLOPs) and contract each
edge's x_src against its unique's theta.

Layout: unique "slots" tiled 128/PSUM-tile, grouped into count classes
c=1..CMAX (uniques with more edges are split across slots), so every
tile needs exactly c vector-engine contraction passes over its theta.
All 8 cores share one static schedule (classes padded to 8x128 slots);
tiles of different classes are interleaved so tensor-heavy (c=1) and
vector-heavy (c=4) stretches smooth each other out.

Per tile: 4*nb fp8 DoubleRow matmuls (K=1024) -> theta in PSUM;
ScalarE transposes theta to SBUF bf16 o-major [p, o, 34] (slot 32 is
set to 1.0, the theta-bias lane). Per pass, one DVE 2x tensor_tensor
multiplies theta by a host-streamed o-major x-replica tile whose slot
32 carries acc0 = x@Bb (the theta-bias fold); GpSimd folds slots
[17:33] into [0:16]; one DVE tensor_reduce over [0:17] emits msg.

Stage A (the tiny K=16 GEMM) runs on HOST over unique values only;
h = relu(ea_u@Wa+ba) uploads as fp8 in matmul lhsT layout.

Two launches (layer 1, layer 2); segment-mean node aggregation,
graph pooling and the FC head run on host (trivial FLOPs).
"""
import numpy as np

import sys
for p in ("/opt/trn_rl_repo",):
    if p not in sys.path:
        sys.path.append(p)

import ml_dtypes

from concourse import bass, bacc, mybir, tile
from concourse import bass_utils

E = 100000
N = 50000
NG = 2000
F_IN = 32
EF = 16
H = 32
H2 = 64
NC = 8
CMAX = 4               # max contraction passes per theta tile
NSL = 34               # prod/x-replica slots: 32 products + acc0 + pad

_F32 = mybir.dt.float32
_BF16 = mybir.dt.bfloat16
_F8 = mybir.dt.float8e4
_COPY = mybir.ActivationFunctionType.Copy
_MUL = mybir.AluOpType.mult
_ADD = mybir.AluOpType.add
_DR = mybir.MatmulPerfMode.DoubleRow

_NC_CACHE = {}
LAST_RUNS = []  # BassKernelResults of the device launches in the last kernel() call

BF16 = ml_dtypes.bfloat16
F8E4 = ml_dtypes.float8_e4m3fn


def _build_pass(fo, tile_classes):
    """One GNN layer. fo: per-edge output width (32 / 64). tile_classes:
    per-tile pass count, kernel iteration order (same on all cores)."""
    fi = 32
    d = fi * fo                    # theta width: 1024 / 2048
    nb = d // 512                  # PSUM banks per theta tile: 2 / 4
    ipb = 512 // fo                # i-values per PSUM bank: 16 / 8
    ntile = len(tile_classes)
    S = ntile * 128                # unique slots per core
    Q = sum(tile_classes)          # contraction passes per core

    nc = bacc.Bacc(None, target_bir_lowering=False)

    hT_d = nc.dram_tensor("hT", [128, 8 * S], _F8, kind="ExternalInput")
    Wb_d = nc.dram_tensor("Wb", [128, 8 * d], _F8, kind="ExternalInput")
    xrp_d = nc.dram_tensor("xrp", [128, Q * fo * NSL], _BF16,
                           kind="ExternalInput")
    msg_d = nc.dram_tensor("msg", [128, Q * fo], _BF16, kind="ExternalOutput")

    with tile.TileContext(nc) as tc:
        with (
            tc.tile_pool(name="w", bufs=1) as wpool,
            tc.tile_pool(name="ths", bufs=4) as thspool,
            tc.tile_pool(name="xr", bufs=12) as xrpool,
            tc.tile_pool(name="pr", bufs=4) as prpool,
            tc.tile_pool(name="th", bufs=3 if nb == 2 else 2,
                         space=bass.MemorySpace.PSUM) as thpool,
        ):
            # weights + first h chunk land before the bulk h stream
            wb = wpool.tile([128, 4, nb, 2, 512], _F8)
            nc.sync.dma_start(wb[:].rearrange("p a b c e -> p (a b c e)"), Wb_d[:])
            hT = wpool.tile([128, 8, S], _F8)
            CH0 = min(512, S)
            nc.sync.dma_start(hT[:, :, :CH0], hT_d[:].rearrange(
                "p (j s) -> p j s", j=8)[:, :, :CH0])
            for a in range(CH0, S, 2048):
                b = min(a + 2048, S)
                nc.sync.dma_start(hT[:, :, a:b], hT_d[:].rearrange(
                    "p (j s) -> p j s", j=8)[:, :, a:b])
            msg = wpool.tile([128, Q * fo], _BF16)

            # pass-deferred reduce: run the reduce of pass q-1 after the
            # multiply of pass q has been issued, so the in-order DVE
            # queue never waits on the GpSimd fold.
            pending = []

            def flush_reduce():
                if not pending:
                    return
                prod_p, q_p = pending.pop()
                with nc.allow_low_precision(reason="fp32 internal accum"):
                    nc.vector.tensor_reduce(
                        msg[:, q_p * fo:(q_p + 1) * fo],
                        prod_p[:, :, 0:17], mybir.AxisListType.X, _ADD)

            dumped = [0]
            q = 0
            for tg, c in enumerate(tile_classes):
                th = thpool.tile([128, d // fo, fo], _F32,
                                 name="th", tag="th")
                for b in range(nb):
                    out_b = th[:, b * ipb:(b + 1) * ipb, :] \
                        .rearrange("p i o -> p (i o)")
                    for jp in range(4):
                        nc.tensor.matmul(
                            out_b,
                            hT[:, 2 * jp:2 * jp + 2, tg * 128:(tg + 1) * 128],
                            wb[:, jp, b, :, :],
                            start=(jp == 0), stop=(jp == 3),
                            perf_mode=_DR,
                        )
                ths = thspool.tile([128, fo, NSL], _BF16)
                # o-major transpose PSUM->SBUF; slot 32 = 1.0 (bias lane,
                # multiplies acc0 in the x-replica); slot 33 never read.
                nc.scalar.activation(
                    ths[:, :, 0:fi].rearrange("p o i -> p i o"),
                    th[:].rearrange("p i o -> p (i o)"), _COPY)
                nc.scalar.activation(
                    ths[:, :, 32:33], wb[:, 0, 0, 0, 0:fo].unsqueeze(2),
                    _COPY, scale=0.0, bias=1.0)
                for _j in range(c):
                    xr = xrpool.tile([128, fo, NSL], _BF16)
                    nc.sync.dma_start(
                        xr[:].rearrange("p o s -> p (o s)"),
                        xrp_d[:, q * fo * NSL:(q + 1) * fo * NSL])
                    prod = prpool.tile([128, fo, NSL], _BF16)
                    nc.vector.tensor_tensor(
                        prod[:, :, 0:33], ths[:, :, 0:33], xr[:, :, 0:33],
                        _MUL)
                    nc.gpsimd.tensor_tensor(
                        prod[:, :, 0:16], prod[:, :, 0:16],
                        prod[:, :, 17:33], _ADD)
                    flush_reduce()
                    pending.append((prod, q))
                    q += 1

                if (tg + 1) % 8 == 0 or tg + 1 == ntile:
                    while pending:
                        flush_reduce()
                    e = q * fo
                    if e > dumped[0]:
                        nc.sync.dma_start(
                            msg_d[:, dumped[0]:e], msg[:, dumped[0]:e])
                        dumped[0] = e

    nc.compile()
    return nc


def _get_nc(fo, tile_classes):
    key = (fo, tile_classes)
    if key not in _NC_CACHE:
        _NC_CACHE[key] = _build_pass(fo, tile_classes)
    return _NC_CACHE[key]


def _relu(v):
    return np.maximum(v, 0.0)


class _SegMean:
    """Sort-based segment mean (np.add.at is too slow)."""

    def __init__(self, idx, n):
        self.n = n
        self.order = np.argsort(idx, kind="stable")
        sorted_idx = np.asarray(idx)[self.order]
        self.uniq, self.starts = np.unique(sorted_idx, return_index=True)
        self.cnt = np.maximum(
            np.bincount(np.asarray(idx), minlength=n), 1.0
        ).astype(np.float32)[:, None]

    def __call__(self, vals):
        out = np.zeros((self.n, vals.shape[1]), np.float32)
        out[self.uniq] = np.add.reduceat(vals[self.order], self.starts, axis=0)
        return out / self.cnt


class _Schedule:
    """Dedup schedule shared by both layers.

    tile_classes: per-tile pass count in emission order (same all cores)
    slot_uid: [NC, S] unique-row index per slot (-1 = pad)
    eid:      [NC, 128, Q] edge id per (partition, pass) (-1 = pad)
    """

    def __init__(self, eap):
        v = (eap[:, 0].astype(np.int64) << 8) | eap[:, 1].astype(np.int64)
        uniq, inv, counts = np.unique(v, return_inverse=True,
                                      return_counts=True)
        self.uniq_vals = uniq
        order = np.argsort(inv, kind="stable").astype(np.int64)
        starts = np.zeros(len(uniq) + 1, np.int64)
        np.cumsum(counts, out=starts[1:])

        cls_uid = {c: [] for c in range(1, CMAX + 1)}
        cls_eid = {c: [] for c in range(1, CMAX + 1)}
        for c in range(1, CMAX + 1):
            us = np.where(counts == c)[0]
            if len(us):
                cls_uid[c].append(us)
                cls_eid[c].append(order[starts[us][:, None] + np.arange(c)])
        big = np.where(counts > CMAX)[0]
        extra_uid = {c: [] for c in range(1, CMAX + 1)}
        extra_eid = {c: [] for c in range(1, CMAX + 1)}
        for u in big:
            k = int(counts[u])
            s = int(starts[u])
            while k > 0:
                c = min(k, CMAX)
                extra_uid[c].append(u)
                extra_eid[c].append(order[s:s + c])
                s += c
                k -= c
        for c in range(1, CMAX + 1):
            if extra_uid[c]:
                cls_uid[c].append(np.asarray(extra_uid[c], np.int64))
                cls_eid[c].append(np.stack(extra_eid[c]))

        # per class: pad to NC*128 slots, deal round-robin to cores
        cl_uid = {}     # c -> [n, NC, 128]
        cl_eid = {}     # c -> [n, NC, 128, c]
        n_of = {}
        for c in range(1, CMAX + 1):
            if not cls_uid[c]:
                continue
            uid = np.concatenate(cls_uid[c])
            eid = np.concatenate(cls_eid[c]).reshape(-1, c)
            Sc = len(uid)
            n = -(-Sc // (NC * 128))
            pad = n * NC * 128 - Sc
            uid = np.concatenate([uid, np.full(pad, -1, np.int64)])
            eid = np.concatenate([eid, np.full((pad, c), -1, np.int64)])
            cl_uid[c] = uid.reshape(n, NC, 128)
            cl_eid[c] = eid.reshape(n, NC, 128, c)
            n_of[c] = n

        # interleave classes: tile k of class c sits at fraction (k+.5)/n_c
        emission = sorted(
            ((k + 0.5) / n_of[c], c, k) for c in n_of for k in range(n_of[c]))
        self.tile_classes = tuple(c for _, c, k in emission)

        core_uid = [[] for _ in range(NC)]
        core_eid = [[] for _ in range(NC)]
        for _, c, kidx in emission:
            for k in range(NC):
                core_uid[k].append(cl_uid[c][kidx, k, :])
                core_eid[k].append(cl_eid[c][kidx, k, :, :].T)  # [c, 128]
        self.slot_uid = np.stack([np.concatenate(u) for u in core_uid])
        # eid per core: passes in emission order -> [128, Q]
        self.eid = np.stack(
            [np.concatenate(e, axis=0).T for e in core_eid])
        self.ntile = len(self.tile_classes)
        self.Q = sum(self.tile_classes)
        self.valid = self.eid >= 0
        self.eid0 = np.maximum(self.eid, 0)


def _run_pass(fo, sch, h_u, xfull, Bb):
    """One GNN layer on device. h_u: [U, 1024] fp32 unique hidden
    activations; xfull: [E, 32] fp32 per-edge source features."""
    nc = _get_nc(fo, sch.tile_classes)
    ac0_full = (xfull @ Bb.reshape(32, fo)).astype(BF16)
    h_u8 = h_u.astype(F8E4)
    U = h_u8.shape[0]

    in_maps = []
    for k in range(NC):
        uid = sch.slot_uid[k]
        hs = h_u8[np.minimum(uid, U - 1)]
        hs[uid < 0] = 0
        S = hs.shape[0]
        hT = np.ascontiguousarray(
            hs.T.reshape(8, 128, S).transpose(1, 0, 2).reshape(128, 8 * S))
        xs = xfull[sch.eid0[k]].astype(BF16)           # [128, Q, 32]
        xr = np.zeros((128, sch.Q, fo, NSL), BF16)
        xr[:, :, :, 0:32] = xs[:, :, None, :]
        xr[:, :, :, 32] = ac0_full[sch.eid0[k]]
        in_maps.append(dict(hT=hT, Wb=_WB_CACHE[fo],
                            xrp=xr.reshape(128, -1)))

    res = bass_utils.run_bass_kernel_spmd(nc, in_maps, core_ids=list(range(NC)))
    LAST_RUNS.append(res)

    msg_full = np.zeros((E, fo), np.float32)
    for k in range(NC):
        m = np.asarray(res.results[k]["msg"]).astype(np.float32)
        m = m.reshape(128, sch.Q, fo)
        vm = sch.valid[k]
        msg_full[sch.eid[k][vm]] = m[vm]
    return msg_full


_WB_CACHE = {}


def _pack_wb(fo, Wb):
    # [k=1024, d] -> [p, jp, bank, plane, n]; k = (2*jp+plane)*128+p
    d = 32 * fo
    nb = d // 512
    _WB_CACHE[fo] = np.ascontiguousarray(
        Wb.reshape(4, 2, 128, nb, 512).transpose(2, 0, 3, 1, 4)
        .reshape(128, 8 * d)).astype(F8E4)


def kernel(**inputs):
    x = np.asarray(inputs["x"], np.float32)
    edge_index = np.asarray(inputs["edge_index"])
    eap = np.asarray(inputs["edge_attr_packed"])
    batch = np.asarray(inputs["batch"])
    W1a = np.asarray(inputs["W1a"], np.float32)
    W1b = np.asarray(inputs["W1b"], np.float32)
    W2a = np.asarray(inputs["W2a"], np.float32)
    W2b = np.asarray(inputs["W2b"], np.float32)
    b1a = np.asarray(inputs["b1a"], np.float32)
    b1b = np.asarray(inputs["b1b"], np.float32)
    b2a = np.asarray(inputs["b2a"], np.float32)
    b2b = np.asarray(inputs["b2b"], np.float32)
    root1 = np.asarray(inputs["root1"], np.float32)
    bias1 = np.asarray(inputs["bias1"], np.float32)
    root2 = np.asarray(inputs["root2"], np.float32)
    bias2 = np.asarray(inputs["bias2"], np.float32)

    LAST_RUNS.clear()
    sch = _Schedule(eap)
    _pack_wb(32, W1b)
    _pack_wb(64, W2b)

    # unique edge-attr bit patterns -> [U, 16] (MSB-first per byte)
    shifts = np.arange(15, -1, -1, dtype=np.int64)
    ea_u = ((sch.uniq_vals[:, None] >> shifts) & 1).astype(np.float32)

    src, dst = edge_index[0], edge_index[1]
    segmean_dst = _SegMean(dst, N)

    h1_u = _relu(ea_u @ W1a + b1a)
    msg1 = _run_pass(32, sch, h1_u, x[src], b1b)
    h = _relu(segmean_dst(msg1) + x @ root1 + bias1)

    h2_u = _relu(ea_u @ W2a + b2a)
    msg2 = _run_pass(64, sch, h2_u, h[src], b2b)
    h = _relu(segmean_dst(msg2) + h @ root2 + bias2)

    g = _SegMean(batch, NG)(h)
    g = _relu(g @ np.asarray(inputs["fcW1"], np.float32) + np.asarray(inputs["fcb1"], np.float32))
    g = _relu(g @ np.asarray(inputs["fcW2"], np.float32) + np.asarray(inputs["fcb2"], np.float32))
    g = _relu(g @ np.asarray(inputs["fcW3"], np.float32) + np.asarray(inputs["fcb3"], np.float32))
    return (g @ np.asarray(inputs["fcW4"], np.float32) + np.asarray(inputs["fcb4"], np.float32)).astype(np.float32)


# revision 9
# speedup vs baseline: 1.0536x; 1.0536x over previous
"""GCN-with-edge-features kernel for 8 Trainium2 cores.

Strategy: the per-edge weight matrices theta = relu(ea@Wa+ba)@Wb+bb
depend ONLY on the 16-bit edge attribute, and E=100k random edges hit
only ~51.3k distinct attribute values. We compute theta once per
UNIQUE attribute (0.51x the dominant GEMM FLOPs) and contract each
edge's x_src against its unique's theta.

Layout: unique "slots" tiled 128/PSUM-tile, grouped into count classes
c=1..CMAX (uniques with more edges are split across slots), so every
tile needs exactly c vector-engine contraction passes over its theta.
All 8 cores share one static schedule (classes padded to 8x128 slots);
tiles of different classes are interleaved so tensor-heavy (c=1) and
vector-heavy (c=4) stretches smooth each other out.

Per tile: 4*nb fp8 DoubleRow matmuls (K=1024) -> theta in PSUM;
ScalarE copies theta to SBUF bf16 i-major [p, 33, fo] (contiguous; row
32 is set to 1.0, the theta-bias lane). Per pass, one DVE 2x
tensor_tensor multiplies theta by a host-streamed x-replica tile whose
row 32 carries acc0 = x@Bb (the theta-bias fold); GpSimd folds rows
[17:33] into [0:16]; one DVE tensor_reduce over a strided o-major view
of rows [0:17] emits msg (reduce is 1x regardless of strides).

Stage A (the tiny K=16 GEMM) runs on HOST over unique values only;
h = relu(ea_u@Wa+ba) uploads as fp8 in matmul lhsT layout.

Two launches (layer 1, layer 2); segment-mean node aggregation,
graph pooling and the FC head run on host (trivial FLOPs).
"""
import numpy as np

import sys
for p in ("/opt/trn_rl_repo",):
    if p not in sys.path:
        sys.path.append(p)

import ml_dtypes

from concourse import bass, bacc, mybir, tile
from concourse import bass_utils

E = 100000
N = 50000
NG = 2000
F_IN = 32
EF = 16
H = 32
H2 = 64
NC = 8
CMAX = 4               # max contraction passes per theta tile
NSL = 33               # prod/x-replica i-rows: 32 products + acc0 lane

_F32 = mybir.dt.float32
_BF16 = mybir.dt.bfloat16
_F8 = mybir.dt.float8e4
_COPY = mybir.ActivationFunctionType.Copy
_MUL = mybir.AluOpType.mult
_ADD = mybir.AluOpType.add
_DR = mybir.MatmulPerfMode.DoubleRow

_NC_CACHE = {}
LAST_RUNS = []  # BassKernelResults of the device launches in the last kernel() call

BF16 = ml_dtypes.bfloat16
F8E4 = ml_dtypes.float8_e4m3fn


def _build_pass(fo, tile_classes):
    """One GNN layer. fo: per-edge output width (32 / 64). tile_classes:
    per-tile pass count, kernel iteration order (same on all cores)."""
    fi = 32
    d = fi * fo                    # theta width: 1024 / 2048
    nb = d // 512                  # PSUM banks per theta tile: 2 / 4
    ipb = 512 // fo                # i-values per PSUM bank: 16 / 8
    ntile = len(tile_classes)
    S = ntile * 128                # unique slots per core
    Q = sum(tile_classes)          # contraction passes per core

    nc = bacc.Bacc(None, target_bir_lowering=False)

    hT_d = nc.dram_tensor("hT", [128, 8 * S], _F8, kind="ExternalInput")
    Wb_d = nc.dram_tensor("Wb", [128, 8 * d], _F8, kind="ExternalInput")
    xrp_d = nc.dram_tensor("xrp", [128, Q * fo * NSL], _BF16,
                           kind="ExternalInput")
    msg_d = nc.dram_tensor("msg", [128, Q * fo], _BF16, kind="ExternalOutput")

    with tile.TileContext(nc) as tc:
        with (
            tc.tile_pool(name="w", bufs=1) as wpool,
            tc.tile_pool(name="ths", bufs=4) as thspool,
            tc.tile_pool(name="xr", bufs=12) as xrpool,
            tc.tile_pool(name="pr", bufs=4) as prpool,
            tc.tile_pool(name="th", bufs=3 if nb == 2 else 2,
                         space=bass.MemorySpace.PSUM) as thpool,
        ):
            # weights + first h chunk land before the bulk h stream
            wb = wpool.tile([128, 4, nb, 2, 512], _F8)
            nc.sync.dma_start(wb[:].rearrange("p a b c e -> p (a b c e)"), Wb_d[:])
            hT = wpool.tile([128, 8, S], _F8)
            CH0 = min(512, S)
            nc.sync.dma_start(hT[:, :, :CH0], hT_d[:].rearrange(
                "p (j s) -> p j s", j=8)[:, :, :CH0])
            for a in range(CH0, S, 2048):
                b = min(a + 2048, S)
                nc.sync.dma_start(hT[:, :, a:b], hT_d[:].rearrange(
                    "p (j s) -> p j s", j=8)[:, :, a:b])
            msg = wpool.tile([128, Q * fo], _BF16)

            # pass-deferred reduce: run the reduce of pass q-1 after the
            # multiply of pass q has been issued, so the in-order DVE
            # queue never waits on the GpSimd fold.
            pending = []

            def flush_reduce():
                if not pending:
                    return
                prod_p, q_p = pending.pop()
                with nc.allow_low_precision(reason="fp32 internal accum"):
                    nc.vector.tensor_reduce(
                        msg[:, q_p * fo:(q_p + 1) * fo],
                        prod_p[:, 0:17, :].rearrange("p i o -> p o i"),
                        mybir.AxisListType.X, _ADD)

            dumped = [0]
            q = 0
            for tg, c in enumerate(tile_classes):
                th = thpool.tile([128, d // fo, fo], _F32,
                                 name="th", tag="th")
                for b in range(nb):
                    out_b = th[:, b * ipb:(b + 1) * ipb, :] \
                        .rearrange("p i o -> p (i o)")
                    for jp in range(4):
                        nc.tensor.matmul(
                            out_b,
                            hT[:, 2 * jp:2 * jp + 2, tg * 128:(tg + 1) * 128],
                            wb[:, jp, b, :, :],
                            start=(jp == 0), stop=(jp == 3),
                            perf_mode=_DR,
                        )
                ths = thspool.tile([128, NSL, fo], _BF16)
                # contiguous PSUM->SBUF bf16 copy; row 32 = 1.0 (bias
                # lane, multiplies acc0 in the x-replica).
                nc.scalar.activation(
                    ths[:, 0:fi, :].rearrange("p i o -> p (i o)"),
                    th[:].rearrange("p i o -> p (i o)"), _COPY)
                nc.scalar.activation(
                    ths[:, 32:33, :], wb[:, 0, 0, 0, 0:fo].unsqueeze(1),
                    _COPY, scale=0.0, bias=1.0)
                for _j in range(c):
                    xr = xrpool.tile([128, NSL, fo], _BF16)
                    nc.sync.dma_start(
                        xr[:].rearrange("p s o -> p (s o)"),
                        xrp_d[:, q * fo * NSL:(q + 1) * fo * NSL])
                    prod = prpool.tile([128, NSL, fo], _BF16)
                    nc.vector.tensor_tensor(prod[:], ths[:], xr[:], _MUL)
                    nc.gpsimd.tensor_tensor(
                        prod[:, 0:16, :], prod[:, 0:16, :],
                        prod[:, 17:33, :], _ADD)
                    flush_reduce()
                    pending.append((prod, q))
                    q += 1

                if (tg + 1) % 8 == 0 or tg + 1 == ntile:
                    while pending:
                        flush_reduce()
                    e = q * fo
                    if e > dumped[0]:
                        nc.sync.dma_start(
                            msg_d[:, dumped[0]:e], msg[:, dumped[0]:e])
                        dumped[0] = e

    nc.compile()
    return nc


def _get_nc(fo, tile_classes):
    key = (fo, tile_classes)
    if key not in _NC_CACHE:
        _NC_CACHE[key] = _build_pass(fo, tile_classes)
    return _NC_CACHE[key]


def _relu(v):
    return np.maximum(v, 0.0)


class _SegMean:
    """Sort-based segment mean (np.add.at is too slow)."""

    def __init__(self, idx, n):
        self.n = n
        self.order = np.argsort(idx, kind="stable")
        sorted_idx = np.asarray(idx)[self.order]
        self.uniq, self.starts = np.unique(sorted_idx, return_index=True)
        self.cnt = np.maximum(
            np.bincount(np.asarray(idx), minlength=n), 1.0
        ).astype(np.float32)[:, None]

    def __call__(self, vals):
        out = np.zeros((self.n, vals.shape[1]), np.float32)
        out[self.uniq] = np.add.reduceat(vals[self.order], self.starts, axis=0)
        return out / self.cnt


class _Schedule:
    """Dedup schedule shared by both layers.

    tile_classes: per-tile pass count in emission order (same all cores)
    slot_uid: [NC, S] unique-row index per slot (-1 = pad)
    eid:      [NC, 128, Q] edge id per (partition, pass) (-1 = pad)
    """

    def __init__(self, eap):
        v = (eap[:, 0].astype(np.int64) << 8) | eap[:, 1].astype(np.int64)
        uniq, inv, counts = np.unique(v, return_inverse=True,
                                      return_counts=True)
        self.uniq_vals = uniq
        order = np.argsort(inv, kind="stable").astype(np.int64)
        starts = np.zeros(len(uniq) + 1, np.int64)
        np.cumsum(counts, out=starts[1:])

        cls_uid = {c: [] for c in range(1, CMAX + 1)}
        cls_eid = {c: [] for c in range(1, CMAX + 1)}
        for c in range(1, CMAX + 1):
            us = np.where(counts == c)[0]
            if len(us):
                cls_uid[c].append(us)
                cls_eid[c].append(order[starts[us][:, None] + np.arange(c)])
        big = np.where(counts > CMAX)[0]
        extra_uid = {c: [] for c in range(1, CMAX + 1)}
        extra_eid = {c: [] for c in range(1, CMAX + 1)}
        for u in big:
            k = int(counts[u])
            s = int(starts[u])
            while k > 0:
                c = min(k, CMAX)
                extra_uid[c].append(u)
                extra_eid[c].append(order[s:s + c])
                s += c
                k -= c
        for c in range(1, CMAX + 1):
            if extra_uid[c]:
                cls_uid[c].append(np.asarray(extra_uid[c], np.int64))
                cls_eid[c].append(np.stack(extra_eid[c]))

        # per class: pad to NC*128 slots, deal round-robin to cores
        cl_uid = {}     # c -> [n, NC, 128]
        cl_eid = {}     # c -> [n, NC, 128, c]
        n_of = {}
        for c in range(1, CMAX + 1):
            if not cls_uid[c]:
                continue
            uid = np.concatenate(cls_uid[c])
            eid = np.concatenate(cls_eid[c]).reshape(-1, c)
            Sc = len(uid)
            n = -(-Sc // (NC * 128))
            pad = n * NC * 128 - Sc
            uid = np.concatenate([uid, np.full(pad, -1, np.int64)])
            eid = np.concatenate([eid, np.full((pad, c), -1, np.int64)])
            cl_uid[c] = uid.reshape(n, NC, 128)
            cl_eid[c] = eid.reshape(n, NC, 128, c)
            n_of[c] = n

        # interleave classes: tile k of class c sits at fraction (k+.5)/n_c
        emission = sorted(
            ((k + 0.5) / n_of[c], c, k) for c in n_of for k in range(n_of[c]))
        self.tile_classes = tuple(c for _, c, k in emission)

        core_uid = [[] for _ in range(NC)]
        core_eid = [[] for _ in range(NC)]
        for _, c, kidx in emission:
            for k in range(NC):
                core_uid[k].append(cl_uid[c][kidx, k, :])
                core_eid[k].append(cl_eid[c][kidx, k, :, :].T)  # [c, 128]
        self.slot_uid = np.stack([np.concatenate(u) for u in core_uid])
        # eid per core: passes in emission order -> [128, Q]
        self.eid = np.stack(
            [np.concatenate(e, axis=0).T for e in core_eid])
        self.ntile = len(self.tile_classes)
        self.Q = sum(self.tile_classes)
        self.valid = self.eid >= 0
        self.eid0 = np.maximum(self.eid, 0)


def _run_pass(fo, sch, h_u, xfull, Bb):
    """One GNN layer on device. h_u: [U, 1024] fp32 unique hidden
    activations; xfull: [E, 32] fp32 per-edge source features."""
    nc = _get_nc(fo, sch.tile_classes)
    ac0_full = (xfull @ Bb.reshape(32, fo)).astype(BF16)
    h_u8 = h_u.astype(F8E4)
    U = h_u8.shape[0]

    in_maps = []
    for k in range(NC):
        uid = sch.slot_uid[k]
        hs = h_u8[np.minimum(uid, U - 1)]
        hs[uid < 0] = 0
        S = hs.shape[0]
        hT = np.ascontiguousarray(
            hs.T.reshape(8, 128, S).transpose(1, 0, 2).reshape(128, 8 * S))
        xs = xfull[sch.eid0[k]].astype(BF16)           # [128, Q, 32]
        xr = np.empty((128, sch.Q, NSL, fo), BF16)
        xr[:, :, 0:32, :] = xs[:, :, :, None]
        xr[:, :, 32, :] = ac0_full[sch.eid0[k]]
        in_maps.append(dict(hT=hT, Wb=_WB_CACHE[fo],
                            xrp=xr.reshape(128, -1)))

    res = bass_utils.run_bass_kernel_spmd(nc, in_maps, core_ids=list(range(NC)))
    LAST_RUNS.append(res)

    msg_full = np.zeros((E, fo), np.float32)
    for k in range(NC):
        m = np.asarray(res.results[k]["msg"]).astype(np.float32)
        m = m.reshape(128, sch.Q, fo)
        vm = sch.valid[k]
        msg_full[sch.eid[k][vm]] = m[vm]
    return msg_full


_WB_CACHE = {}


def _pack_wb(fo, Wb):
    # [k=1024, d] -> [p, jp, bank, plane, n]; k = (2*jp+plane)*128+p
    d = 32 * fo
    nb = d // 512
    _WB_CACHE[fo] = np.ascontiguousarray(
        Wb.reshape(4, 2, 128, nb, 512).transpose(2, 0, 3, 1, 4)
        .reshape(128, 8 * d)).astype(F8E4)


def kernel(**inputs):
    x = np.asarray(inputs["x"], np.float32)
    edge_index = np.asarray(inputs["edge_index"])
    eap = np.asarray(inputs["edge_attr_packed"])
    batch = np.asarray(inputs["batch"])
    W1a = np.asarray(inputs["W1a"], np.float32)
    W1b = np.asarray(inputs["W1b"], np.float32)
    W2a = np.asarray(inputs["W2a"], np.float32)
    W2b = np.asarray(inputs["W2b"], np.float32)
    b1a = np.asarray(inputs["b1a"], np.float32)
    b1b = np.asarray(inputs["b1b"], np.float32)
    b2a = np.asarray(inputs["b2a"], np.float32)
    b2b = np.asarray(inputs["b2b"], np.float32)
    root1 = np.asarray(inputs["root1"], np.float32)
    bias1 = np.asarray(inputs["bias1"], np.float32)
    root2 = np.asarray(inputs["root2"], np.float32)
    bias2 = np.asarray(inputs["bias2"], np.float32)

    LAST_RUNS.clear()
    sch = _Schedule(eap)
    _pack_wb(32, W1b)
    _pack_wb(64, W2b)

    # unique edge-attr bit patterns -> [U, 16] (MSB-first per byte)
    shifts = np.arange(15, -1, -1, dtype=np.int64)
    ea_u = ((sch.uniq_vals[:, None] >> shifts) & 1).astype(np.float32)

    src, dst = edge_index[0], edge_index[1]
    segmean_dst = _SegMean(dst, N)

    h1_u = _relu(ea_u @ W1a + b1a)
    msg1 = _run_pass(32, sch, h1_u, x[src], b1b)
    h = _relu(segmean_dst(msg1) + x @ root1 + bias1)

    h2_u = _relu(ea_u @ W2a + b2a)
    msg2 = _run_pass(64, sch, h2_u, h[src], b2b)
    h = _relu(segmean_dst(msg2) + h @ root2 + bias2)

    g = _SegMean(batch, NG)(h)
    g = _relu(g @ np.asarray(inputs["fcW1"], np.float32) + np.asarray(inputs["fcb1"], np.float32))
    g = _relu(g @ np.asarray(inputs["fcW2"], np.float32) + np.asarray(inputs["fcb2"], np.float32))
    g = _relu(g @ np.asarray(inputs["fcW3"], np.float32) + np.asarray(inputs["fcb3"], np.float32))
    return (g @ np.asarray(inputs["fcW4"], np.float32) + np.asarray(inputs["fcb4"], np.float32)).astype(np.float32)


# revision 10
# speedup vs baseline: 2.5428x; 2.4134x over previous
"""GCN-with-edge-features kernel for 8 Trainium2 cores.

The per-edge weight matrices theta = relu(ea@Wa+ba)@Wb+bb depend ONLY
on the 16-bit edge attribute, and E=100k random edges hit only ~51.3k
distinct values. The device computes theta once per UNIQUE attribute
(0.51x the dominant GEMM FLOPs) as pure fp8 DoubleRow matmul work:

  per 128-unique tile: 4*nb DR matmuls (K=1024, N=512) -> PSUM,
  ScalarE compresses fp32 -> bf16 SBUF, DMA streams theta to HBM.

All three stages pipeline under the tensor engine (scalar 2.0us, DMA
1.5us vs matmul 3.6us per L2 tile), so each launch runs at the fp8
matmul roofline. Stage A of the edge-net (K=16 GEMM over unique attrs,
~2 GFLOP) runs on host, uploading h = relu(ea_u@Wa+ba) as fp8 in
matmul lhsT layout; replicated Wb uploads once per layer.

The per-edge contraction msg[e] = x[src[e]] @ theta[u(e)] is only
0.4 GFLOP total (800x less than the GEMMs) and runs on host as
count-class-batched matmuls over zero-copy theta views (unique slots
are emitted in count-sorted order), like the segment-mean aggregation,
graph pooling and FC head already do.

Two launches (layer 1, layer 2).
"""
import numpy as np

import sys
for p in ("/opt/trn_rl_repo",):
    if p not in sys.path:
        sys.path.append(p)

import ml_dtypes

from concourse import bass, bacc, mybir, tile
from concourse import bass_utils

E = 100000
N = 50000
NG = 2000
F_IN = 32
EF = 16
H = 32
H2 = 64
NC = 8

_F32 = mybir.dt.float32
_BF16 = mybir.dt.bfloat16
_F8 = mybir.dt.float8e4
_COPY = mybir.ActivationFunctionType.Copy
_DR = mybir.MatmulPerfMode.DoubleRow

_NC_CACHE = {}
_WB_CACHE = {}
LAST_RUNS = []  # BassKernelResults of the device launches in the last kernel() call

BF16 = ml_dtypes.bfloat16
F8E4 = ml_dtypes.float8_e4m3fn


def _build_pass(fo, ntile):
    """One GNN layer: theta for ntile*128 unique slots per core.
    fo: per-edge output width (32 / 64)."""
    d = 32 * fo                    # theta width: 1024 / 2048
    nb = d // 512                  # PSUM banks per theta tile: 2 / 4
    ipb = 512 // fo                # i-values per PSUM bank: 16 / 8
    S = ntile * 128

    nc = bacc.Bacc(None, target_bir_lowering=False)
    hT_d = nc.dram_tensor("hT", [128, 8 * S], _F8, kind="ExternalInput")
    Wb_d = nc.dram_tensor("Wb", [128, 8 * d], _F8, kind="ExternalInput")
    tho_d = nc.dram_tensor("tho", [128, ntile * d], _BF16,
                           kind="ExternalOutput")

    with tile.TileContext(nc) as tc:
        with (
            tc.tile_pool(name="w", bufs=1) as wpool,
            tc.tile_pool(name="ths", bufs=4) as thspool,
            tc.tile_pool(name="th", bufs=3 if nb == 2 else 2,
                         space=bass.MemorySpace.PSUM) as thpool,
        ):
            # weights + first h chunk land before the bulk h stream
            wb = wpool.tile([128, 4, nb, 2, 512], _F8)
            nc.sync.dma_start(wb[:].rearrange("p a b c e -> p (a b c e)"), Wb_d[:])
            hT = wpool.tile([128, 8, S], _F8)
            CH0 = min(512, S)
            nc.sync.dma_start(hT[:, :, :CH0], hT_d[:].rearrange(
                "p (j s) -> p j s", j=8)[:, :, :CH0])
            for a in range(CH0, S, 2048):
                b = min(a + 2048, S)
                nc.sync.dma_start(hT[:, :, a:b], hT_d[:].rearrange(
                    "p (j s) -> p j s", j=8)[:, :, a:b])

            for tg in range(ntile):
                th = thpool.tile([128, d], _F32, name="th", tag="th")
                for b in range(nb):
                    for jp in range(4):
                        nc.tensor.matmul(
                            th[:, b * 512:(b + 1) * 512],
                            hT[:, 2 * jp:2 * jp + 2, tg * 128:(tg + 1) * 128],
                            wb[:, jp, b, :, :],
                            start=(jp == 0), stop=(jp == 3),
                            perf_mode=_DR,
                        )
                ths = thspool.tile([128, d], _BF16)
                nc.scalar.activation(ths[:], th[:], _COPY)
                nc.sync.dma_start(tho_d[:, tg * d:(tg + 1) * d], ths[:])

    nc.compile()
    return nc


def _get_nc(fo, ntile):
    key = (fo, ntile)
    if key not in _NC_CACHE:
        _NC_CACHE[key] = _build_pass(fo, ntile)
    return _NC_CACHE[key]


def _relu(v):
    return np.maximum(v, 0.0)


class _SegMean:
    """Sort-based segment mean (np.add.at is too slow)."""

    def __init__(self, idx, n):
        self.n = n
        self.order = np.argsort(idx, kind="stable")
        sorted_idx = np.asarray(idx)[self.order]
        self.uniq, self.starts = np.unique(sorted_idx, return_index=True)
        self.cnt = np.maximum(
            np.bincount(np.asarray(idx), minlength=n), 1.0
        ).astype(np.float32)[:, None]

    def __call__(self, vals):
        out = np.zeros((self.n, vals.shape[1]), np.float32)
        out[self.uniq] = np.add.reduceat(vals[self.order], self.starts, axis=0)
        return out / self.cnt


class _Schedule:
    """Dedup bookkeeping shared by both layers.

    Unique slots are count-sorted so each count class is a contiguous
    slot range; the device emits theta in slot order, letting the host
    contraction take zero-copy per-class theta views.
    """

    def __init__(self, eap):
        v = (eap[:, 0].astype(np.int64) << 8) | eap[:, 1].astype(np.int64)
        uniq, inv, counts = np.unique(v, return_inverse=True,
                                      return_counts=True)
        self.uniq_vals = uniq
        U = len(uniq)
        edge_order = np.argsort(inv, kind="stable").astype(np.int64)
        starts = np.zeros(U + 1, np.int64)
        np.cumsum(counts, out=starts[1:])

        order_u = np.argsort(counts, kind="stable")   # uniques by count
        sc = counts[order_u]
        self.ntile = -(-U // (NC * 128))
        S = self.ntile * NC * 128
        slot_uid = np.concatenate([order_u, np.full(S - U, -1, np.int64)])
        # slot s -> (tile s//1024, core (s//128)%8, partition s%128)
        self.core_uid = slot_uid.reshape(self.ntile, NC, 128).transpose(1, 0, 2) \
            .reshape(NC, -1)

        # per count class: slot range [lo, hi) and edge-id matrix [n, c]
        self.classes = []
        lo = 0
        for c in np.unique(sc):
            hi = int(np.searchsorted(sc, c, side="right"))
            us = order_u[lo:hi]
            em = edge_order[starts[us][:, None] + np.arange(c)]
            self.classes.append((int(c), lo, hi, em))
            lo = hi
        self.S = S

    def contract(self, theta_slots, xfull, Bb, fo):
        """msg[e] = xfull[e] @ theta[u(e)] + xfull[e] @ Bb."""
        msg = xfull @ Bb.reshape(32, fo)
        for c, lo, hi, em in self.classes:
            out = np.matmul(xfull[em], theta_slots[lo:hi])  # [n, c, fo]
            msg[em.reshape(-1)] += out.reshape(-1, fo)
        return msg


def _run_pass(fo, sch, h_u, xfull, Bb):
    """One GNN layer. h_u: [U, 1024] fp32 unique hidden activations;
    xfull: [E, 32] fp32 per-edge source features."""
    nc = _get_nc(fo, sch.ntile)
    d = 32 * fo
    h_u8 = h_u.astype(F8E4)
    U = h_u8.shape[0]

    in_maps = []
    for k in range(NC):
        uid = sch.core_uid[k]
        hs = h_u8[np.minimum(uid, U - 1)]
        hs[uid < 0] = 0
        S = hs.shape[0]
        hT = np.ascontiguousarray(
            hs.T.reshape(8, 128, S).transpose(1, 0, 2).reshape(128, 8 * S))
        in_maps.append(dict(hT=hT, Wb=_WB_CACHE[fo]))

    res = bass_utils.run_bass_kernel_spmd(nc, in_maps, core_ids=list(range(NC)))
    LAST_RUNS.append(res)

    # [core][128, ntile*d] -> [S, 32, fo] fp32 in slot order
    theta = np.empty((sch.ntile, NC, 128, 32, fo), np.float32)
    for k in range(NC):
        m = np.asarray(res.results[k]["tho"]).astype(np.float32)
        theta[:, k] = m.reshape(128, sch.ntile, 32, fo).transpose(1, 0, 2, 3)
    theta_slots = theta.reshape(sch.S, 32, fo)

    return sch.contract(theta_slots, xfull, Bb, fo)


def _pack_wb(fo, Wb):
    # [k=1024, d] -> [p, jp, bank, plane, n]; k = (2*jp+plane)*128+p
    d = 32 * fo
    nb = d // 512
    _WB_CACHE[fo] = np.ascontiguousarray(
        Wb.reshape(4, 2, 128, nb, 512).transpose(2, 0, 3, 1, 4)
        .reshape(128, 8 * d)).astype(F8E4)


def kernel(**inputs):
    x = np.asarray(inputs["x"], np.float32)
    edge_index = np.asarray(inputs["edge_index"])
    eap = np.asarray(inputs["edge_attr_packed"])
    batch = np.asarray(inputs["batch"])
    W1a = np.asarray(inputs["W1a"], np.float32)
    W1b = np.asarray(inputs["W1b"], np.float32)
    W2a = np.asarray(inputs["W2a"], np.float32)
    W2b = np.asarray(inputs["W2b"], np.float32)
    b1a = np.asarray(inputs["b1a"], np.float32)
    b1b = np.asarray(inputs["b1b"], np.float32)
    b2a = np.asarray(inputs["b2a"], np.float32)
    b2b = np.asarray(inputs["b2b"], np.float32)
    root1 = np.asarray(inputs["root1"], np.float32)
    bias1 = np.asarray(inputs["bias1"], np.float32)
    root2 = np.asarray(inputs["root2"], np.float32)
    bias2 = np.asarray(inputs["bias2"], np.float32)

    LAST_RUNS.clear()
    sch = _Schedule(eap)
    _pack_wb(32, W1b)
    _pack_wb(64, W2b)

    # unique edge-attr bit patterns -> [U, 16] (MSB-first per byte)
    shifts = np.arange(15, -1, -1, dtype=np.int64)
    ea_u = ((sch.uniq_vals[:, None] >> shifts) & 1).astype(np.float32)

    src, dst = edge_index[0], edge_index[1]
    segmean_dst = _SegMean(dst, N)

    h1_u = _relu(ea_u @ W1a + b1a)
    msg1 = _run_pass(32, sch, h1_u, x[src], b1b)
    h = _relu(segmean_dst(msg1) + x @ root1 + bias1)

    h2_u = _relu(ea_u @ W2a + b2a)
    msg2 = _run_pass(64, sch, h2_u, h[src], b2b)
    h = _relu(segmean_dst(msg2) + h @ root2 + bias2)

    g = _SegMean(batch, NG)(h)
    g = _relu(g @ np.asarray(inputs["fcW1"], np.float32) + np.asarray(inputs["fcb1"], np.float32))
    g = _relu(g @ np.asarray(inputs["fcW2"], np.float32) + np.asarray(inputs["fcb2"], np.float32))
    g = _relu(g @ np.asarray(inputs["fcW3"], np.float32) + np.asarray(inputs["fcb3"], np.float32))
    return (g @ np.asarray(inputs["fcW4"], np.float32) + np.asarray(inputs["fcb4"], np.float32)).astype(np.float32)


# revision 17
# speedup vs baseline: 2.5614x; 1.0073x over previous
"""GCN-with-edge-features kernel for 8 Trainium2 cores.

The per-edge weight matrices theta = relu(ea@Wa+ba)@Wb+bb depend ONLY
on the 16-bit edge attribute, and E=100k random edges hit only ~51.3k
distinct values. The device computes theta once per UNIQUE attribute
(0.51x the dominant GEMM FLOPs) as pure fp8 DoubleRow matmul work:

  per 128-unique tile: 4*nb DR matmuls (K=1024, N=512) -> PSUM,
  ScalarE compresses fp32 -> bf16 SBUF, DMA streams theta to HBM.

All three stages pipeline under the tensor engine (scalar 2.0us, DMA
1.5us vs matmul 3.6us per L2 tile), so each launch runs at the fp8
matmul roofline. Stage A of the edge-net (K=16 GEMM over unique attrs,
~2 GFLOP) runs on host, uploading h = relu(ea_u@Wa+ba) as fp8 in
matmul lhsT layout; replicated Wb uploads once per layer.

The per-edge contraction msg[e] = x[src[e]] @ theta[u(e)] is only
0.4 GFLOP total (800x less than the GEMMs) and runs on host as
count-class-batched matmuls over zero-copy theta views (unique slots
are emitted in count-sorted order), like the segment-mean aggregation,
graph pooling and FC head already do.

theta2 depends only on the edge attributes (not on layer-1 output), so
BOTH layers' theta GEMMs run in a single launch.
"""
import numpy as np

import sys
for p in ("/opt/trn_rl_repo",):
    if p not in sys.path:
        sys.path.append(p)

import ml_dtypes

from concourse import bass, bacc, mybir, tile
from concourse import bass_utils

E = 100000
N = 50000
NG = 2000
F_IN = 32
EF = 16
H = 32
H2 = 64
NC = 8

_F32 = mybir.dt.float32
_BF16 = mybir.dt.bfloat16
_F8 = mybir.dt.float8e4
_COPY = mybir.ActivationFunctionType.Copy
_DR = mybir.MatmulPerfMode.DoubleRow

_NC_CACHE = {}
_WB_CACHE = {}
LAST_RUNS = []  # BassKernelResults of the device launches in the last kernel() call

BF16 = ml_dtypes.bfloat16
F8E4 = ml_dtypes.float8_e4m3fn


def _build_pass(ntile):
    """Both GNN layers' theta GEMMs in one launch: theta1 [S, 1024] then
    theta2 [S, 2048] for S = ntile*128 unique slots per core. theta2
    depends only on the edge attributes, not on layer-1 output, so both
    layers share one kernel (one fill + drain instead of two)."""
    S = ntile * 128

    nc = bacc.Bacc(None, target_bir_lowering=False)
    hT1_d = nc.dram_tensor("hT1", [128, 8 * S], _F8, kind="ExternalInput")
    hT2_d = nc.dram_tensor("hT2", [128, 8 * S], _F8, kind="ExternalInput")
    Wb1_d = nc.dram_tensor("Wb1", [128, 8 * 1024], _F8, kind="ExternalInput")
    Wb2_d = nc.dram_tensor("Wb2", [128, 8 * 2048], _F8, kind="ExternalInput")
    tho1_d = nc.dram_tensor("tho1", [128, ntile * 1024], _BF16,
                            kind="ExternalOutput")
    tho2_d = nc.dram_tensor("tho2", [128, ntile * 2048], _BF16,
                            kind="ExternalOutput")

    with tile.TileContext(nc) as tc:
        with (
            tc.tile_pool(name="w", bufs=1) as wpool,
            tc.tile_pool(name="ths", bufs=6) as thspool,
            tc.tile_pool(name="th", bufs=2,
                         space=bass.MemorySpace.PSUM) as thpool,
        ):
            # layer-1 weights + first h1 chunk land first; the bulk h
            # streams ride the (otherwise idle) GpSimd DMA queue so
            # per-tile theta stores never queue behind them.
            wb1 = wpool.tile([128, 4, 2, 2, 512], _F8)
            nc.sync.dma_start(wb1[:].rearrange("p a b c e -> p (a b c e)"),
                              Wb1_d[:])
            hT1 = wpool.tile([128, 8, S], _F8)
            nc.sync.dma_start(hT1[:, :, :128], hT1_d[:].rearrange(
                "p (j s) -> p j s", j=8)[:, :, :128])
            hT2 = wpool.tile([128, 8, S], _F8)
            for a in range(128, S, 1024):
                b = min(a + 1024, S)
                nc.gpsimd.dma_start(hT1[:, :, a:b], hT1_d[:].rearrange(
                    "p (j s) -> p j s", j=8)[:, :, a:b])
            wb2 = wpool.tile([128, 4, 4, 2, 512], _F8)
            nc.gpsimd.dma_start(wb2[:].rearrange("p a b c e -> p (a b c e)"),
                                Wb2_d[:])
            for a in range(0, S, 1024):
                b = min(a + 1024, S)
                nc.gpsimd.dma_start(hT2[:, :, a:b], hT2_d[:].rearrange(
                    "p (j s) -> p j s", j=8)[:, :, a:b])

            for fo, hT, wb, tho_d in ((32, hT1, wb1, tho1_d),
                                      (64, hT2, wb2, tho2_d)):
                d = 32 * fo
                nb = d // 512
                for tg in range(ntile):
                    th = thpool.tile([128, 2048], _F32, name="th", tag="th")
                    for b in range(nb):
                        for jp in range(4):
                            nc.tensor.matmul(
                                th[:, b * 512:(b + 1) * 512],
                                hT[:, 2 * jp:2 * jp + 2,
                                   tg * 128:(tg + 1) * 128],
                                wb[:, jp, b, :, :],
                                start=(jp == 0), stop=(jp == 3),
                                perf_mode=_DR,
                            )
                    ths = thspool.tile([128, 2048], _BF16)
                    nc.scalar.activation(ths[:, :d], th[:, :d], _COPY)
                    nc.sync.dma_start(tho_d[:, tg * d:(tg + 1) * d],
                                      ths[:, :d])

    nc.compile()
    return nc


def _get_nc(ntile):
    if ntile not in _NC_CACHE:
        _NC_CACHE[ntile] = _build_pass(ntile)
    return _NC_CACHE[ntile]


def _relu(v):
    return np.maximum(v, 0.0)


class _SegMean:
    """Sort-based segment mean (np.add.at is too slow)."""

    def __init__(self, idx, n):
        self.n = n
        self.order = np.argsort(idx, kind="stable")
        sorted_idx = np.asarray(idx)[self.order]
        self.uniq, self.starts = np.unique(sorted_idx, return_index=True)
        self.cnt = np.maximum(
            np.bincount(np.asarray(idx), minlength=n), 1.0
        ).astype(np.float32)[:, None]

    def __call__(self, vals):
        out = np.zeros((self.n, vals.shape[1]), np.float32)
        out[self.uniq] = np.add.reduceat(vals[self.order], self.starts, axis=0)
        return out / self.cnt


class _Schedule:
    """Dedup bookkeeping shared by both layers.

    Unique slots are count-sorted so each count class is a contiguous
    slot range; the device emits theta in slot order, letting the host
    contraction take zero-copy per-class theta views.
    """

    def __init__(self, eap):
        v = (eap[:, 0].astype(np.int64) << 8) | eap[:, 1].astype(np.int64)
        uniq, inv, counts = np.unique(v, return_inverse=True,
                                      return_counts=True)
        self.uniq_vals = uniq
        U = len(uniq)
        edge_order = np.argsort(inv, kind="stable").astype(np.int64)
        starts = np.zeros(U + 1, np.int64)
        np.cumsum(counts, out=starts[1:])

        order_u = np.argsort(counts, kind="stable")   # uniques by count
        sc = counts[order_u]
        self.ntile = -(-U // (NC * 128))
        S = self.ntile * NC * 128
        slot_uid = np.concatenate([order_u, np.full(S - U, -1, np.int64)])
        # slot s -> (tile s//1024, core (s//128)%8, partition s%128)
        self.core_uid = slot_uid.reshape(self.ntile, NC, 128).transpose(1, 0, 2) \
            .reshape(NC, -1)

        # per count class: slot range [lo, hi) and edge-id matrix [n, c]
        self.classes = []
        lo = 0
        for c in np.unique(sc):
            hi = int(np.searchsorted(sc, c, side="right"))
            us = order_u[lo:hi]
            em = edge_order[starts[us][:, None] + np.arange(c)]
            self.classes.append((int(c), lo, hi, em))
            lo = hi
        self.S = S

    def contract(self, theta_slots, xfull, Bb, fo):
        """msg[e] = xfull[e] @ theta[u(e)] + xfull[e] @ Bb."""
        msg = xfull @ Bb.reshape(32, fo)
        for c, lo, hi, em in self.classes:
            out = np.matmul(xfull[em], theta_slots[lo:hi])  # [n, c, fo]
            msg[em.reshape(-1)] += out.reshape(-1, fo)
        return msg


def _pack_hT(sch, h_u):
    """[U, 1024] fp32 -> per-core fp8 lhsT layout [128, 8*S]."""
    h_u8 = h_u.astype(F8E4)
    U = h_u8.shape[0]
    outs = []
    for k in range(NC):
        uid = sch.core_uid[k]
        hs = h_u8[np.minimum(uid, U - 1)]
        hs[uid < 0] = 0
        S = hs.shape[0]
        outs.append(np.ascontiguousarray(
            hs.T.reshape(8, 128, S).transpose(1, 0, 2).reshape(128, 8 * S)))
    return outs


def _theta_slots(sch, res, name, fo):
    """[core][128, ntile*d] bf16 -> [S, 32, fo] fp32 in slot order."""
    theta = np.empty((sch.ntile, NC, 128, 32, fo), np.float32)
    for k in range(NC):
        m = np.asarray(res.results[k][name]).astype(np.float32)
        theta[:, k] = m.reshape(128, sch.ntile, 32, fo).transpose(1, 0, 2, 3)
    return theta.reshape(sch.S, 32, fo)


def _run_both(sch, h1_u, h2_u):
    """One launch computing theta1 and theta2 for all unique slots."""
    nc = _get_nc(sch.ntile)
    hT1 = _pack_hT(sch, h1_u)
    hT2 = _pack_hT(sch, h2_u)
    in_maps = [dict(hT1=hT1[k], hT2=hT2[k], Wb1=_WB_CACHE[32],
                    Wb2=_WB_CACHE[64]) for k in range(NC)]
    res = bass_utils.run_bass_kernel_spmd(nc, in_maps, core_ids=list(range(NC)))
    LAST_RUNS.append(res)
    return res


def _pack_wb(fo, Wb):
    # [k=1024, d] -> [p, jp, bank, plane, n]; k = (2*jp+plane)*128+p
    d = 32 * fo
    nb = d // 512
    _WB_CACHE[fo] = np.ascontiguousarray(
        Wb.reshape(4, 2, 128, nb, 512).transpose(2, 0, 3, 1, 4)
        .reshape(128, 8 * d)).astype(F8E4)


def kernel(**inputs):
    x = np.asarray(inputs["x"], np.float32)
    edge_index = np.asarray(inputs["edge_index"])
    eap = np.asarray(inputs["edge_attr_packed"])
    batch = np.asarray(inputs["batch"])
    W1a = np.asarray(inputs["W1a"], np.float32)
    W1b = np.asarray(inputs["W1b"], np.float32)
    W2a = np.asarray(inputs["W2a"], np.float32)
    W2b = np.asarray(inputs["W2b"], np.float32)
    b1a = np.asarray(inputs["b1a"], np.float32)
    b1b = np.asarray(inputs["b1b"], np.float32)
    b2a = np.asarray(inputs["b2a"], np.float32)
    b2b = np.asarray(inputs["b2b"], np.float32)
    root1 = np.asarray(inputs["root1"], np.float32)
    bias1 = np.asarray(inputs["bias1"], np.float32)
    root2 = np.asarray(inputs["root2"], np.float32)
    bias2 = np.asarray(inputs["bias2"], np.float32)

    LAST_RUNS.clear()
    sch = _Schedule(eap)
    _pack_wb(32, W1b)
    _pack_wb(64, W2b)

    # unique edge-attr bit patterns -> [U, 16] (MSB-first per byte)
    shifts = np.arange(15, -1, -1, dtype=np.int64)
    ea_u = ((sch.uniq_vals[:, None] >> shifts) & 1).astype(np.float32)

    src, dst = edge_index[0], edge_index[1]
    segmean_dst = _SegMean(dst, N)

    h1_u = _relu(ea_u @ W1a + b1a)
    h2_u = _relu(ea_u @ W2a + b2a)
    res = _run_both(sch, h1_u, h2_u)

    theta1 = _theta_slots(sch, res, "tho1", 32)
    msg1 = sch.contract(theta1, x[src], b1b, 32)
    h = _relu(segmean_dst(msg1) + x @ root1 + bias1)

    theta2 = _theta_slots(sch, res, "tho2", 64)
    msg2 = sch.contract(theta2, h[src], b2b, 64)
    h = _relu(segmean_dst(msg2) + h @ root2 + bias2)

    g = _SegMean(batch, NG)(h)
    g = _relu(g @ np.asarray(inputs["fcW1"], np.float32) + np.asarray(inputs["fcb1"], np.float32))
    g = _relu(g @ np.asarray(inputs["fcW2"], np.float32) + np.asarray(inputs["fcb2"], np.float32))
    g = _relu(g @ np.asarray(inputs["fcW3"], np.float32) + np.asarray(inputs["fcb3"], np.float32))
    return (g @ np.asarray(inputs["fcW4"], np.float32) + np.asarray(inputs["fcb4"], np.float32)).astype(np.float32)
